# revision 7
# baseline (speedup 1.0000x reference)
"""Low-rank linear: out = x @ (U @ V)^T = (x @ V^T) @ U^T on 8 TRN2 cores.

Shapes (hardcoded per problem spec):
  x [4, 2048, 4096] f32 -> flat [8192, 4096], row-sharded 1024 rows/core
  U [4096, 64] f32 (replicated), V [64, 4096] f32 (replicated)
  out [4, 2048, 4096] f32

Per-core dataflow:
  hT = V @ x_c^T   (GEMM1: lhsT = V^T k-chunks [128,64], rhs = x^T tiles)
  out_c = hT^T @ U^T (GEMM2: lhsT = hT row-blocks [64,128], rhs = U^T [64,4096])
x^T tiles are produced on-chip via PE transpose (fp32 has no DMA-transpose).
"""

import sys

for p in ("/opt/trn_rl_repo",):
    if p not in sys.path:
        sys.path.insert(0, p)

import numpy as np

import concourse.bass as bass
import concourse.bacc as bacc_mod
import concourse.mybir as mybir
import concourse.tile as tile
from concourse.bass_utils import run_bass_kernel_spmd
from concourse.masks import make_identity

N_CORES = 8
BATCH, SEQ, IN_F = 4, 2048, 4096
ROWS = BATCH * SEQ           # 8192
ROWS_PC = ROWS // N_CORES    # 1024 rows per core
RANK = 64
OUT_F = 4096

P = 128                      # partition dim / k-chunk
N_KC = IN_F // P             # 32 k-chunks
SB = 256                     # rows per super-block (matmul free dim, >=256 for f32r fast path)
N_SB = ROWS_PC // SB         # 4
N_RB = SB // P               # 2 row-blocks per super-block
NB = 512                     # out-feature block (one PSUM bank of fp32)
N_NB = OUT_F // NB           # 8

F32 = mybir.dt.float32
# float32r engages the TRN2 fp32 fast matmul path (1 cycle/row at free-dim
# >=256 vs 4 cycles/row for plain fp32), at tf32-like multiply precision.
MM_DT = mybir.dt.float32r


# Tiles consumed by matmuls must carry MM_DT itself: the BIR verifier
# requires f32r matmul operands to be *rounded to f32r by their producer*
# (the PSUM->SBUF copy does the cast), not just bitcast-viewed.


def build_bass():
    nc = bacc_mod.Bacc("TRN2")
    x_d = nc.declare_dram_parameter("x", [ROWS_PC, IN_F], F32, isOutput=False)
    u_d = nc.declare_dram_parameter("U", [OUT_F, RANK], F32, isOutput=False)
    v_d = nc.declare_dram_parameter("V", [RANK, IN_F], F32, isOutput=False)
    o_d = nc.declare_dram_parameter("out", [ROWS_PC, OUT_F], F32, isOutput=True)

    with tile.TileContext(nc) as tc:
        with (
            tc.tile_pool(name="const", bufs=1) as const,
            tc.tile_pool(name="stage", bufs=3) as stage_p,
            tc.tile_pool(name="xt", bufs=2) as xt_p,
            tc.tile_pool(name="ht", bufs=2) as ht_p,
            tc.tile_pool(name="obuf", bufs=2) as obuf_p,
            tc.tile_pool(name="pt", bufs=2, space="PSUM") as pt_p,
            tc.tile_pool(name="ph", bufs=2, space="PSUM") as ph_p,
            tc.tile_pool(name="po", bufs=4, space="PSUM") as po_p,
        ):
            ident = const.tile([P, P], F32)
            make_identity(nc, ident[:])

            # ---- setup: V^T chunks [128k, 64r] and U^T [64r, 4096o] ----
            vt = const.tile([P, N_KC, RANK], MM_DT, tag="vt")
            v_nat = stage_p.tile([RANK, IN_F], F32, tag="stage")
            nc.sync.dma_start(out=v_nat[:], in_=v_d[:])
            for kc in range(N_KC):
                ps = pt_p.tile([P, RANK], F32, tag="pt")
                # [64r, 128k] -> [128k, 64r]
                nc.tensor.transpose(ps[:], v_nat[:, kc * P : (kc + 1) * P], ident[:RANK, :RANK])
                nc.vector.tensor_copy(out=vt[:, kc, :], in_=ps[:])

            ut = const.tile([RANK, OUT_F], MM_DT, tag="ut")
            u_nat = stage_p.tile([P, N_KC, RANK], F32, tag="stage")
            nc.sync.dma_start(
                out=u_nat[:], in_=u_d[:].rearrange("(n p) r -> p n r", p=P)
            )
            for oc in range(N_KC):  # 32 chunks of 128 out-features
                ps = pt_p.tile([RANK, P], F32, tag="pt")
                # [128o, 64r] -> [64r, 128o]
                nc.tensor.transpose(ps[:], u_nat[:, oc, :], ident[:])
                nc.vector.tensor_copy(out=ut[:, oc * P : (oc + 1) * P], in_=ps[:])

            # ---- main loop over row super-blocks ----
            for sb in range(N_SB):
                xt = xt_p.tile([P, N_KC, SB], MM_DT, tag="xt")
                for rb in range(N_RB):
                    row0 = sb * SB + rb * P
                    xstg = stage_p.tile([P, IN_F], F32, tag="stage")
                    nc.sync.dma_start(out=xstg[:], in_=x_d[row0 : row0 + P, :])
                    for kc in range(N_KC):
                        ps = pt_p.tile([P, P], F32, tag="pt")
                        nc.tensor.transpose(
                            ps[:], xstg[:, kc * P : (kc + 1) * P], ident[:]
                        )
                        if kc % 2 == 0:
                            nc.vector.tensor_copy(
                                out=xt[:, kc, rb * P : (rb + 1) * P], in_=ps[:]
                            )
                        else:
                            nc.scalar.copy(
                                out=xt[:, kc, rb * P : (rb + 1) * P], in_=ps[:]
                            )

                # GEMM1: hT[64, SB] = V @ x_sb^T, accumulated over 32 k-chunks
                ph = ph_p.tile([RANK, SB], F32, tag="ph")
                for kc in range(N_KC):
                    nc.tensor.matmul(
                        ph[:],
                        vt[:, kc, :],
                        xt[:, kc, :],
                        start=(kc == 0),
                        stop=(kc == N_KC - 1),
                    )
                ht = ht_p.tile([RANK, SB], MM_DT, tag="ht")
                nc.vector.tensor_copy(out=ht[:], in_=ph[:])

                # GEMM2: out rows = hT^T @ U^T
                for rb in range(N_RB):
                    row0 = sb * SB + rb * P
                    ob = obuf_p.tile([P, OUT_F], F32, tag="obuf")
                    for nb in range(N_NB):
                        po = po_p.tile([P, NB], F32, tag="po")
                        nc.tensor.matmul(
                            po[:],
                            ht[:, rb * P : (rb + 1) * P],
                            ut[:, nb * NB : (nb + 1) * NB],
                            start=True,
                            stop=True,
                        )
                        if nb % 2 == 0:
                            nc.vector.tensor_copy(
                                out=ob[:, nb * NB : (nb + 1) * NB], in_=po[:]
                            )
                        else:
                            nc.scalar.copy(
                                out=ob[:, nb * NB : (nb + 1) * NB], in_=po[:]
                            )
                    nc.sync.dma_start(out=o_d[row0 : row0 + P, :], in_=ob[:])

    return nc


_NC_CACHE = None


def _get_nc():
    global _NC_CACHE
    if _NC_CACHE is None:
        _NC_CACHE = build_bass()
        _NC_CACHE.finalize()
    return _NC_CACHE


def run(inputs, trace=False):
    """Returns (full_output, exec_time_ns or None)."""
    x = np.ascontiguousarray(np.asarray(inputs["x"], dtype=np.float32))
    u = np.ascontiguousarray(np.asarray(inputs["U"], dtype=np.float32))
    v = np.ascontiguousarray(np.asarray(inputs["V"], dtype=np.float32))
    xf = x.reshape(ROWS, IN_F)

    nc = _get_nc()
    core_ids = list(range(N_CORES))
    in_maps = [
        {"x": xf[c * ROWS_PC : (c + 1) * ROWS_PC], "U": u, "V": v}
        for c in core_ids
    ]
    res = run_bass_kernel_spmd(nc, in_maps, core_ids, trace=trace)
    out = np.concatenate([np.asarray(r["out"]) for r in res.results], axis=0)
    return out.reshape(BATCH, SEQ, OUT_F), res.exec_time_ns


def kernel(**inputs):
    return run(inputs)[0]


# revision 10
# speedup vs baseline: 1.2942x; 1.2942x over previous
"""Low-rank linear: out = x @ (U @ V)^T = (x @ V^T) @ U^T on 8 TRN2 cores.

Shapes (hardcoded per problem spec):
  x [4, 2048, 4096] f32 -> flat [8192, 4096], row-sharded 1024 rows/core
  U [4096, 64] f32 (replicated), V [64, 4096] f32 (replicated)
  out [4, 2048, 4096] f32

Per-core dataflow (3-stage software pipeline over 256-row super-blocks):
  stage T:  PE-transpose x tiles (fp32 has no DMA transpose)
  stage G1: hT[64,256] += VT[kc].T @ xT[kc]  (32 k-chunks, PSUM accumulate)
  stage G2: out rows = hT slices.T @ UT      (8 x 512-wide blocks per 128 rows)
The three stages of consecutive super-blocks are interleaved
instruction-by-instruction on the PE so the HAM clock gate sees real
matmul activity continuously (transpose-mode alone does not count as
PE-busy and lets the PE re-throttle to 1.2 GHz).
"""

import sys

for p in ("/opt/trn_rl_repo",):
    if p not in sys.path:
        sys.path.insert(0, p)

import numpy as np

import concourse.bass as bass
import concourse.bacc as bacc_mod
import concourse.mybir as mybir
import concourse.tile as tile
from concourse.bass_utils import run_bass_kernel_spmd
from concourse.masks import make_identity

N_CORES = 8
BATCH, SEQ, IN_F = 4, 2048, 4096
ROWS = BATCH * SEQ           # 8192
ROWS_PC = ROWS // N_CORES    # 1024 rows per core
RANK = 64
OUT_F = 4096

P = 128                      # partition dim / k-chunk
N_KC = IN_F // P             # 32 k-chunks
SB = 256                     # rows per super-block (>=256 for the f32r fast path)
N_SB = ROWS_PC // SB         # 4
N_RB = SB // P               # 2 row-blocks per super-block
NB = 512                     # out-feature block (one PSUM bank of fp32)
N_NB = OUT_F // NB           # 8
KG = 4                       # k-chunks transposed into one shared PSUM bank
N_G = N_KC // KG             # 8 groups per super-block

F32 = mybir.dt.float32
# float32r = TRN2 fp32 fast matmul path (1 cycle/row at free-dim >= 256 vs 4
# for plain fp32), tf32-like multiply precision. Operand tiles must be typed
# f32r so the producing copy rounds them (BIR verifier requirement).
MM_DT = mybir.dt.float32r


def build_bass():
    nc = bacc_mod.Bacc("TRN2")
    x_d = nc.declare_dram_parameter("x", [ROWS_PC, IN_F], F32, isOutput=False)
    u_d = nc.declare_dram_parameter("U", [OUT_F, RANK], F32, isOutput=False)
    v_d = nc.declare_dram_parameter("V", [RANK, IN_F], F32, isOutput=False)
    o_d = nc.declare_dram_parameter("out", [ROWS_PC, OUT_F], F32, isOutput=True)

    with tile.TileContext(nc) as tc:
        with (
            tc.tile_pool(name="const", bufs=1) as const,
            tc.tile_pool(name="stage", bufs=3) as stage_p,
            tc.tile_pool(name="xt", bufs=2) as xt_p,
            tc.tile_pool(name="ht", bufs=2) as ht_p,
            tc.tile_pool(name="obuf", bufs=2) as obuf_p,
            tc.tile_pool(name="pt", bufs=3, space="PSUM") as pt_p,
            tc.tile_pool(name="ph", bufs=2, space="PSUM") as ph_p,
            tc.tile_pool(name="po", bufs=3, space="PSUM") as po_p,
        ):
            ident = const.tile([P, P], F32)
            make_identity(nc, ident[:])

            # ---- setup: V^T chunks [128k, 64r] and U^T [64r, 4096o] ----
            vt = const.tile([P, N_KC, RANK], MM_DT, tag="vt")
            v_nat = stage_p.tile([RANK, IN_F], F32, tag="stage")
            nc.sync.dma_start(out=v_nat[:], in_=v_d[:])
            ut = const.tile([RANK, OUT_F], MM_DT, tag="ut")
            u_nat = stage_p.tile([P, N_KC, RANK], F32, tag="stage")
            nc.sync.dma_start(
                out=u_nat[:], in_=u_d[:].rearrange("(n p) r -> p n r", p=P)
            )
            for kc in range(N_KC):
                ps = pt_p.tile([P, KG, P], F32, tag="pt")
                # [64r, 128k] -> [128k, 64r]
                nc.tensor.matmul(
                    ps[:, 0, :RANK],
                    v_nat[:, kc * P : (kc + 1) * P],
                    ident[:RANK, :RANK],
                    is_transpose=True,
                )
                # [128o, 64r] -> [64r, 128o]
                nc.tensor.matmul(
                    ps[:RANK, 1, :], u_nat[:, kc, :], ident[:], is_transpose=True
                )
                nc.vector.tensor_copy(out=vt[:, kc, :], in_=ps[:, 0, :RANK])
                nc.scalar.copy(
                    out=ut[:, kc * P : (kc + 1) * P], in_=ps[:RANK, 1, :]
                )

            # ---- 3-stage pipelined main loop ----
            xt = {}   # live xt tiles per sb
            ph = {}   # live GEMM1 psum per sb
            ht = {}   # live hT tiles per sb
            for step in range(N_SB + 2):
                i_t = step          # super-block being transposed
                i_1 = step - 1      # super-block in GEMM1
                i_2 = step - 2      # super-block in GEMM2

                stages = []
                if i_t < N_SB:
                    xt[i_t] = xt_p.tile([P, N_KC, SB], MM_DT, tag="xt", name=f"xt{i_t}")
                    for rb in range(N_RB):
                        row0 = i_t * SB + rb * P
                        stg = stage_p.tile([P, IN_F], F32, tag="stage")
                        nc.sync.dma_start(out=stg[:], in_=x_d[row0 : row0 + P, :])
                        stages.append(stg)
                if 0 <= i_1 < N_SB:
                    ph[i_1] = ph_p.tile([RANK, SB], F32, tag="ph", name=f"ph{i_1}")
                obs = {}
                if i_2 >= 0:
                    for rb in range(N_RB):
                        obs[rb] = obuf_p.tile([P, OUT_F], F32, tag="obuf", name=f"ob{i_2}_{rb}")

                for g in range(N_G):
                    if i_t < N_SB:
                        for rb in range(N_RB):
                            ps = pt_p.tile([P, KG, P], F32, tag="pt")
                            for j in range(KG):
                                kc = g * KG + j
                                nc.tensor.matmul(
                                    ps[:, j, :],
                                    stages[rb][:, kc * P : (kc + 1) * P],
                                    ident[:],
                                    is_transpose=True,
                                    start=(j == 0),
                                    stop=(j == KG - 1),
                                    skip_group_check=True,
                                )
                            dst = xt[i_t][
                                :, g * KG : (g + 1) * KG, rb * P : (rb + 1) * P
                            ]
                            if (g + rb) % 2 == 0:
                                nc.vector.tensor_copy(out=dst, in_=ps[:])
                            else:
                                nc.scalar.copy(out=dst, in_=ps[:])
                    if 0 <= i_1 < N_SB:
                        for j in range(KG):
                            kc = g * KG + j
                            nc.tensor.matmul(
                                ph[i_1][:],
                                vt[:, kc, :],
                                xt[i_1][:, kc, :],
                                start=(kc == 0),
                                stop=(kc == N_KC - 1),
                                skip_group_check=True,
                            )
                    if i_2 >= 0:
                        for t in range(2):
                            idx = g * 2 + t          # 0..15
                            rb, nb = divmod(idx, N_NB)
                            po = po_p.tile([P, NB], F32, tag="po")
                            nc.tensor.matmul(
                                po[:],
                                ht[i_2][:, rb * P : (rb + 1) * P],
                                ut[:, nb * NB : (nb + 1) * NB],
                                start=True,
                                stop=True,
                            )
                            dst = obs[rb][:, nb * NB : (nb + 1) * NB]
                            if t % 2 == 0:
                                nc.vector.tensor_copy(out=dst, in_=po[:])
                            else:
                                nc.scalar.copy(out=dst, in_=po[:])
                            if nb == N_NB - 1:
                                row0 = i_2 * SB + rb * P
                                nc.sync.dma_start(
                                    out=o_d[row0 : row0 + P, :], in_=obs[rb][:]
                                )

                if 0 <= i_1 < N_SB:
                    ht[i_1] = ht_p.tile([RANK, SB], MM_DT, tag="ht", name=f"ht{i_1}")
                    nc.vector.tensor_copy(out=ht[i_1][:], in_=ph[i_1][:])

    return nc


_NC_CACHE = None


def _get_nc():
    global _NC_CACHE
    if _NC_CACHE is None:
        _NC_CACHE = build_bass()
        _NC_CACHE.finalize()
    return _NC_CACHE


def run(inputs, trace=False):
    """Returns (full_output, exec_time_ns or None)."""
    x = np.ascontiguousarray(np.asarray(inputs["x"], dtype=np.float32))
    u = np.ascontiguousarray(np.asarray(inputs["U"], dtype=np.float32))
    v = np.ascontiguousarray(np.asarray(inputs["V"], dtype=np.float32))
    xf = x.reshape(ROWS, IN_F)

    nc = _get_nc()
    core_ids = list(range(N_CORES))
    in_maps = [
        {"x": xf[c * ROWS_PC : (c + 1) * ROWS_PC], "U": u, "V": v}
        for c in core_ids
    ]
    res = run_bass_kernel_spmd(nc, in_maps, core_ids, trace=trace)
    out = np.concatenate([np.asarray(r["out"]) for r in res.results], axis=0)
    return out.reshape(BATCH, SEQ, OUT_F), res.exec_time_ns


def kernel(**inputs):
    return run(inputs)[0]
